# revision 3
# baseline (speedup 1.0000x reference)
"""Trainium2 Bass kernel for nn_Attention2D (B=8, C=256, H=W=32, 8 heads, d=32).

Strategy: data-parallel over batch, one batch element per NeuronCore (8 cores).

Per-core pipeline (n = H*W = 1024 tokens, head dim d = 32):
  load:   host-packed single-DMA inputs (x pre-cast bf16), issued from
          sync/scalar/gpsimd engines in parallel.
  qkv:    q = (scale*w_q) @ x, k = w_k @ x   ([256,1024] head-major, bf16)
          vT[jc] = x[:, jc]^T @ w_v^T, evacuated via strided cast into
          [v_h(32) | ones(32)] x 8 layout ([128,512] bf16 per j-chunk).
  sim^T:  per (head, j-chunk): matmul(lhsT=k slice [32,128], rhs=q slice
          [32,512]) -> PSUM ring tiles [128,1536] (3 units); 4 heads share
          the PE array via K=32 row groups. Softmax max-subtraction skipped
          (logits ~N(0,0.8), max |sim| ~ 4.8; exp safe in fp32).
  exp:    ACT engine Exp over ring tiles -> bf16 SBUF. ACT is the kernel
          roofline: 8.4M exps/core at 128 lanes / 1.2 GHz + per-inst ovh.
  AV:     per (group, pair): accumulate over j-chunks into one PSUM tile:
            headA: lhsT = vt[:, 64hA:64hA+64] ([v|ones], M=64) -> rows 0:64
                   (rows 0:32 main, 32:64 denominator replicated)
            headB: same at tile_position (0,64) -> rows 64:128
          The ones columns make the softmax denominator free on PE.
  norm:   rc = reciprocal_approx_fast(psum); out = main * rc via
          cross-partition-offset tensor_mul into packed out_allT tiles.
  proj:   incremental per output half: y = w_outT^T @ out_allT + b_out,
          bf16 output.

ACT (exp) is the pacing engine. PE work other than sims (qkv, vT, AV of the
previous group, projection) is emitted as "fillers" run between sim rings so
the PE pipeline never drains while ACT is saturated.
"""

import numpy as np
import ml_dtypes
from collections import deque

B, DIM, H, W = 8, 256, 32, 32
NUM_HEADS = 8
DIM_HEAD = 256
D = DIM_HEAD // NUM_HEADS          # 32 per-head dim
N = H * W                          # 1024 tokens
SCALE = (DIM_HEAD / NUM_HEADS) ** (-0.5)
NCORES = 8

_BF16 = ml_dtypes.bfloat16

_PROGRAM = None  # compiled Bass program cache (one per process)


def build_kernel_body(tc, y_ap, x_ap, wqkvT_ap, woutT_ap, bout_ap, dbg=None):
    """Emit the per-core attention program into TileContext tc.

    DRAM tensors (host-packed, 128-partition layout):
      x_ap:     [128, 2048] bf16  (cols 1024*kc+i = x[128*kc + c, i])
      wqkvT_ap: [128, 1536] bf16  (cols 768*kc+o = w_qkvT[128*kc + c, o];
                                   o: 0:256 q pre-scaled, 256:512 k, 512:768 v)
      woutT_ap: [128, 512]  bf16  (cols 256*t+o = w_outT[128*t + hd, o])
      bout_ap:  [128, 2]    fp32  (col oc = b_out[128*oc + c])
      y_ap:     [256, 1024] bf16 out
    """
    from contextlib import ExitStack
    from concourse import mybir
    from concourse.ap import AP

    nc = tc.nc
    f32 = mybir.dt.float32
    bf16 = mybir.dt.bfloat16

    with ExitStack() as ctx:
        singles = ctx.enter_context(tc.tile_pool(name="singles", bufs=1))
        evac = ctx.enter_context(tc.tile_pool(name="evac", bufs=2))
        exp_pool = ctx.enter_context(tc.tile_pool(name="exp", bufs=24))
        rc_pool = ctx.enter_context(tc.tile_pool(name="rc", bufs=2))
        sim_psum = ctx.enter_context(tc.tile_pool(name="simp", bufs=2, space="PSUM"))
        acc_psum = ctx.enter_context(tc.tile_pool(name="accp", bufs=2, space="PSUM"))

        # ---- phase 0: input DMAs, spread across issue engines ----
        xt = singles.tile([128, 2048], bf16, tag="xt")
        nc.sync.dma_start(out=xt, in_=x_ap)
        wqt = singles.tile([128, 1536], bf16, tag="wqt")
        nc.scalar.dma_start(out=wqt, in_=wqkvT_ap)
        wot = singles.tile([128, 512], bf16, tag="wot")
        nc.gpsimd.dma_start(out=wot, in_=woutT_ap)
        bias2 = singles.tile([128, 2], f32, tag="bias2")
        nc.gpsimd.dma_start(out=bias2, in_=bout_ap)

        def xb(kc):
            return xt[:, 1024 * kc:1024 * (kc + 1)]

        def wq(kc):
            return wqt[:, 768 * kc:768 * (kc + 1)]

        # vt tiles [128, 512]: per head h: cols 64h:64h+32 = v_h, rest 1.0.
        vt = []
        for jc in range(8):
            tv = singles.tile([128, 512], bf16, tag=f"vt_{jc}")
            nc.gpsimd.memset(tv, 1.0)
            vt.append(tv)

        # out_allT: packed final-GEMM rhs, 2 tiles [128, 1024] bf16.
        # tile Q rows 32*hq + d = head 4Q+hq; all 128 rows written by norm.
        out_allT = []
        for t in range(2):
            ta = singles.tile([128, N], bf16, tag=f"oa_{t}")
            out_allT.append(ta)

        qb = [None, None]
        kb = [None, None]
        for kind in ("q", "k"):
            for i in range(2):
                dst = singles.tile([128, N], bf16, tag=f"{kind}b_{i}",
                                   name=f"{kind}b{i}")
                (qb if kind == "q" else kb)[i] = dst

        def emit_qkv_chunk(kind, i, nh):
            # kind 'q' -> wq cols 128i, 'k' -> 256+128i
            col0 = (0 if kind == "q" else 256) + 128 * i
            dst = (qb if kind == "q" else kb)[i]
            ps = acc_psum.tile([128, 512], f32, tag="acc",
                               name=f"qkv_{kind}{i}_{nh}")
            for kc in range(2):
                nc.tensor.matmul(
                    ps,
                    wq(kc)[:, col0:col0 + 128],
                    xb(kc)[:, nh * 512:(nh + 1) * 512],
                    start=(kc == 0),
                    stop=(kc == 1),
                )
            nc.vector.tensor_copy(out=dst[:, nh * 512:(nh + 1) * 512], in_=ps)

        def emit_vt(jc):
            ps = acc_psum.tile([128, 256], f32, tag="acc", name=f"vt_{jc}")
            for kc in range(2):
                nc.tensor.matmul(
                    ps,
                    xb(kc)[:, jc * 128:(jc + 1) * 128],
                    wq(kc)[:, 512:768],
                    start=(kc == 0),
                    stop=(kc == 1),
                )
            vb = vt[jc][:, 0:1]
            out_ap = AP(vb.tensor, vb.offset, [list(vb.ap[0]), [64, 8], [1, 32]])
            pb = ps[:, 0:1]
            in_ap = AP(pb.tensor, pb.offset, [list(pb.ap[0]), [32, 8], [1, 32]])
            nc.vector.tensor_copy(out=out_ap, in_=in_ap)

        # ---- filler machinery: PE work interleaved between sim rings ----
        fillers = deque()

        def run_fillers(budget):
            while fillers and budget > 0:
                cost, fn = fillers.popleft()
                fn()
                budget -= cost

        # ---- sim ring machinery ----
        # groups g=0..3: ih=g//2, Q=g%2. unit w=0..31: pq=w//16,
        # jc=(w%16)//2, which=w%2, hq=2pq+which.
        # global unit u = 32g + w; ring = u//3, slot = u%3.
        exp_slices = {}
        state = {"psum": None, "exp": None, "units": 0}

        def flush_ring():
            if state["psum"] is None:
                return
            w = state["units"] * 512
            nc.scalar.activation(
                out=state["exp"][:, 0:w],
                in_=state["psum"][:, 0:w],
                func=mybir.ActivationFunctionType.Exp,
            )
            state["psum"] = None
            state["exp"] = None
            state["units"] = 0

        def emit_sim_unit(g, w):
            ih, Q = g // 2, g % 2
            pq, jc, which = w // 16, (w % 16) // 2, w % 2
            hq = 2 * pq + which
            u = 32 * g + w
            if state["psum"] is None:
                state["psum"] = sim_psum.tile([128, 1536], f32, tag="sim",
                                              name=f"sim_{u}")
                state["exp"] = exp_pool.tile([128, 1536], bf16, tag="exp",
                                             name=f"exp_{u}")
            s = state["units"]
            tp = (96, 0) if hq == 3 else None
            nc.tensor.matmul(
                state["psum"][:, s * 512:(s + 1) * 512],
                kb[Q][32 * hq:32 * (hq + 1), jc * 128:(jc + 1) * 128],
                qb[Q][32 * hq:32 * (hq + 1), ih * 512:(ih + 1) * 512],
                start=True,
                stop=True,
                tile_position=tp,
            )
            exp_slices[(g, w)] = (state["exp"], s)
            state["units"] += 1
            if state["units"] == 3:
                flush_ring()
                run_fillers(800)

        # ---- AV + normalize ----
        av_psums = {}

        def av_mms(g, pq, jc_lo, jc_hi):
            ih, Q = g // 2, g % 2
            hA, hB = 4 * Q + 2 * pq, 4 * Q + 2 * pq + 1  # global heads
            if (g, pq) not in av_psums:
                av_psums[(g, pq)] = acc_psum.tile([128, 512], f32, tag="acc",
                                                  name=f"av_{g}_{pq}")
            ps = av_psums[(g, pq)]
            for jc in range(jc_lo, jc_hi):
                eA, sA = exp_slices[(g, pq * 16 + jc * 2)]
                eB, sB = exp_slices[(g, pq * 16 + jc * 2 + 1)]
                st, sp = (jc == 0), (jc == 7)
                nc.tensor.matmul(
                    ps[0:64, :], vt[jc][:, 64 * hA:64 * hA + 64],
                    eA[:, sA * 512:(sA + 1) * 512], start=st, stop=sp)
                nc.tensor.matmul(
                    ps[64:128, :], vt[jc][:, 64 * hB:64 * hB + 64],
                    eB[:, sB * 512:(sB + 1) * 512], start=st, stop=sp,
                    tile_position=(0, 64))

        def av_norm(g, pq):
            ih, Q = g // 2, g % 2
            ps = av_psums.pop((g, pq))
            # rows 0:32 mainA, 32:64 denA, 64:96 mainB, 96:128 denB
            rc = rc_pool.tile([128, 512], f32, tag="rc")
            nc.vector.reciprocal_approx_fast(out=rc[:, :], in_=ps[:, :])
            dst = out_allT[Q]
            r0 = 64 * pq
            nc.vector.tensor_mul(
                out=dst[r0:r0 + 32, ih * 512:(ih + 1) * 512],
                in0=ps[0:32, :], in1=rc[32:64, :])
            nc.vector.tensor_mul(
                out=dst[r0 + 32:r0 + 64, ih * 512:(ih + 1) * 512],
                in0=ps[64:96, :], in1=rc[96:128, :])

        def push_av_group(g):
            for pq in range(2):
                for jc0 in range(0, 8, 2):
                    fillers.append(
                        (460, lambda g=g, pq=pq, jc0=jc0:
                         av_mms(g, pq, jc0, jc0 + 2)))
                fillers.append((60, lambda g=g, pq=pq: av_norm(g, pq)))

        def emit_proj(nh, oc, dma_engine):
            ps = acc_psum.tile([128, 512], f32, tag="acc",
                               name=f"proj_{nh}_{oc}")
            for t in range(2):
                nc.tensor.matmul(
                    ps,
                    wot[:, 256 * t + 128 * oc:256 * t + 128 * oc + 128],
                    out_allT[t][:, nh * 512:(nh + 1) * 512],
                    start=(t == 0),
                    stop=(t == 1),
                )
            ys = evac.tile([128, 512], bf16, tag="y")
            nc.vector.tensor_scalar_add(out=ys, in0=ps,
                                        scalar1=bias2[:, oc:oc + 1])
            dma_engine.dma_start(
                out=y_ap[oc * 128:(oc + 1) * 128, nh * 512:(nh + 1) * 512],
                in_=ys,
            )

        # ---- emission schedule ----
        emit_qkv_chunk("k", 0, 0)
        emit_qkv_chunk("q", 0, 0)
        emit_qkv_chunk("k", 0, 1)
        emit_qkv_chunk("q", 0, 1)

        for kind, i, nh in (("k", 1, 0), ("k", 1, 1), ("q", 1, 0), ("q", 1, 1)):
            fillers.append((620, lambda a=kind, b=i, c=nh: emit_qkv_chunk(a, b, c)))
        for jc in range(8):
            fillers.append((320, lambda jc=jc: emit_vt(jc)))

        for g in range(4):
            for w in range(32):
                emit_sim_unit(g, w)
                # once group g's pairA units are all emitted in the last
                # group, its AV can interleave into the remaining rings
                if g == 3 and w == 15:
                    for jc0 in range(0, 8, 2):
                        fillers.append(
                            (460, lambda jc0=jc0: av_mms(3, 0, jc0, jc0 + 2)))
                    fillers.append((60, lambda: av_norm(3, 0)))
            if g < 3:
                push_av_group(g)
            if g == 2:
                # proj of token half 0: out_allT[*][:, 0:512] complete
                # after AV groups 0 (Q0) and 1 (Q1) normalize (ih=0)
                fillers.append((400, lambda: emit_proj(0, 0, nc.sync)))
                fillers.append((400, lambda: emit_proj(0, 1, nc.sync)))
        flush_ring()
        run_fillers(10 ** 9)

        # tail: AV group 3 pairB + proj of token half 1
        av_mms(3, 1, 0, 8)
        av_norm(3, 1)
        emit_proj(1, 0, nc.scalar)
        emit_proj(1, 1, nc.sync)

        if dbg is not None:
            for nm, tile_ in (("qb0", qb[0]), ("kb0", kb[0]), ("vt0", vt[0]),
                              ("oa0", out_allT[0]), ("oa1", out_allT[1])):
                if nm in dbg:
                    nc.sync.dma_start(out=dbg[nm], in_=tile_)
            if "exp0" in dbg:
                et, s = exp_slices[(0, 0)]
                nc.sync.dma_start(out=dbg["exp0"], in_=et[:, s * 512:(s + 1) * 512])


def _prep_weights(w_qkv, w_out, b_out):
    """Host-side weight preparation (numpy)."""
    wq = w_qkv.astype(np.float32).copy()
    wq[0:DIM_HEAD] *= SCALE                      # fold softmax scale into w_q
    wqkvT = np.ascontiguousarray(wq.T)                        # [256, 768]
    wqkvT = np.concatenate([wqkvT[0:128], wqkvT[128:256]], axis=1)  # [128,1536]
    wqkvT = np.ascontiguousarray(wqkvT).astype(_BF16)
    woutT = np.ascontiguousarray(w_out.astype(np.float32).T)  # [256, 256]
    woutT = np.ascontiguousarray(
        np.concatenate([woutT[0:128], woutT[128:256]], axis=1)).astype(_BF16)
    bout = np.ascontiguousarray(
        b_out.astype(np.float32).reshape(2, 128).T)           # [128, 2]
    return wqkvT, woutT, bout


def _build_program():
    global _PROGRAM
    if _PROGRAM is not None:
        return _PROGRAM
    import concourse.tile as tile
    from concourse import bacc, mybir

    nc = bacc.Bacc("TRN2", target_bir_lowering=False, debug=False,
                   num_devices=NCORES)
    x_ap = nc.dram_tensor("x", [128, 2048], mybir.dt.bfloat16,
                          kind="ExternalInput").ap()
    wqkvT_ap = nc.dram_tensor("wqkvT", [128, 1536], mybir.dt.bfloat16,
                              kind="ExternalInput").ap()
    woutT_ap = nc.dram_tensor("woutT", [128, 512], mybir.dt.bfloat16,
                              kind="ExternalInput").ap()
    bout_ap = nc.dram_tensor("bout", [128, 2], mybir.dt.float32,
                             kind="ExternalInput").ap()
    y_ap = nc.dram_tensor("y", [DIM, N], mybir.dt.bfloat16,
                          kind="ExternalOutput").ap()
    with tile.TileContext(nc) as tc:
        build_kernel_body(tc, y_ap, x_ap, wqkvT_ap, woutT_ap, bout_ap)
    nc.compile()
    _PROGRAM = nc
    return nc


def kernel(x, w_qkv, w_out, b_out, trace=False):
    """Full-input entry point: shard over batch, run on 8 cores, gather."""
    from concourse import bass_utils

    nc = _build_program()
    wqkvT, woutT, bout = _prep_weights(w_qkv, w_out, b_out)
    in_maps = []
    for b in range(B):
        xb = np.asarray(x[b], dtype=np.float32).reshape(DIM, N)
        xb = np.ascontiguousarray(
            np.concatenate([xb[0:128], xb[128:256]], axis=1)).astype(_BF16)
        in_maps.append({
            "x": xb,
            "wqkvT": wqkvT,
            "woutT": woutT,
            "bout": bout,
        })
    res = bass_utils.run_bass_kernel_spmd(
        nc, in_maps, core_ids=list(range(NCORES)), trace=trace)
    y = np.stack([
        res.results[b]["y"].astype(np.float32).reshape(DIM, H, W)
        for b in range(B)
    ])
    kernel.last_results = res
    return y


# revision 8
# speedup vs baseline: 1.0554x; 1.0554x over previous
"""Trainium2 Bass kernel for nn_Attention2D (B=8, C=256, H=W=32, 8 heads, d=32).

Strategy: data-parallel over batch, one batch element per NeuronCore (8 cores).

Per-core pipeline (n = H*W = 1024 tokens, head dim d = 32):
  load:   host-packed single-DMA inputs (x pre-cast bf16), issued from
          sync/scalar/gpsimd engines in parallel.
  qkv:    q = (scale*w_q) @ x, k = w_k @ x   ([256,1024] head-major, bf16)
          vT[jc] = x[:, jc]^T @ w_v^T, evacuated via strided cast into
          [v_h(32) | ones(32)] x 8 layout ([128,512] bf16 per j-chunk).
  sim^T:  per (head, j-chunk): matmul(lhsT=k slice [32,128], rhs=q slice
          [32,512]) -> PSUM ring tiles [128,1536] (3 units); 4 heads share
          the PE array via K=32 row groups. Softmax max-subtraction skipped
          (logits ~N(0,0.8), max |sim| ~ 4.8; exp safe in fp32).
  exp:    ACT engine Exp over ring tiles -> bf16 SBUF. ACT is the kernel
          roofline: 8.4M exps/core at 128 lanes / 1.2 GHz + per-inst ovh.
  AV:     per (group, pair): accumulate over j-chunks into one PSUM tile:
            headA: lhsT = vt[:, 64hA:64hA+64] ([v|ones], M=64) -> rows 0:64
                   (rows 0:32 main, 32:64 denominator replicated)
            headB: same at tile_position (0,64) -> rows 64:128
          The ones columns make the softmax denominator free on PE.
  norm:   rc = reciprocal_approx_fast(psum); out = main * rc via
          cross-partition-offset tensor_mul into packed out_allT tiles.
  proj:   incremental per output half: y = w_outT^T @ out_allT + b_out,
          bf16 output.

ACT (exp) is the pacing engine. PE work other than sims (qkv, vT, AV of the
previous group, projection) is emitted in half-group blocks between sim
blocks: the 2-deep PSUM ring keeps ACT busy across each ~3.4us AV block, and
block-contiguous matmuls of one shape avoid PE weight-config thrash (a
per-ring interleave measured ~2x slower matmuls).
"""

import numpy as np
import ml_dtypes
from collections import deque

B, DIM, H, W = 8, 256, 32, 32
NUM_HEADS = 8
DIM_HEAD = 256
D = DIM_HEAD // NUM_HEADS          # 32 per-head dim
N = H * W                          # 1024 tokens
SCALE = (DIM_HEAD / NUM_HEADS) ** (-0.5)
NCORES = 8

_BF16 = ml_dtypes.bfloat16

_PROGRAM = None  # compiled Bass program cache (one per process)


def build_kernel_body(tc, y_ap, x_ap, wqkvT_ap, woutT_ap, bout_ap, dbg=None):
    """Emit the per-core attention program into TileContext tc.

    DRAM tensors (host-packed, 128-partition layout):
      x_ap:     [128, 2048] bf16  (cols 1024*kc+i = x[128*kc + c, i])
      wqkvT_ap: [128, 1536] bf16  (cols 768*kc+o = w_qkvT[128*kc + c, o];
                                   o: 0:256 q pre-scaled, 256:512 k, 512:768 v)
      woutT_ap: [128, 512]  bf16  (cols 256*t+o = w_outT[128*t + hd, o])
      bout_ap:  [128, 2]    fp32  (col oc = b_out[128*oc + c])
      y_ap:     [256, 1024] bf16 out
    """
    from contextlib import ExitStack
    from concourse import mybir
    from concourse.ap import AP

    nc = tc.nc
    f32 = mybir.dt.float32
    bf16 = mybir.dt.bfloat16

    with ExitStack() as ctx:
        singles = ctx.enter_context(tc.tile_pool(name="singles", bufs=1))
        evac = ctx.enter_context(tc.tile_pool(name="evac", bufs=2))
        exp_pool = ctx.enter_context(tc.tile_pool(name="exp", bufs=24))
        rc_pool = ctx.enter_context(tc.tile_pool(name="rc", bufs=2))
        sim_psum = ctx.enter_context(tc.tile_pool(name="simp", bufs=2, space="PSUM"))
        acc_psum = ctx.enter_context(tc.tile_pool(name="accp", bufs=2, space="PSUM"))

        # ---- phase 0: input DMAs, spread across issue engines ----
        xt = singles.tile([128, 2048], bf16, tag="xt")
        nc.sync.dma_start(out=xt, in_=x_ap)
        wqt = singles.tile([128, 1536], bf16, tag="wqt")
        nc.scalar.dma_start(out=wqt, in_=wqkvT_ap)
        wot = singles.tile([128, 512], bf16, tag="wot")
        nc.gpsimd.dma_start(out=wot, in_=woutT_ap)
        bias2 = singles.tile([128, 2], f32, tag="bias2")
        nc.gpsimd.dma_start(out=bias2, in_=bout_ap)

        def xb(kc):
            return xt[:, 1024 * kc:1024 * (kc + 1)]

        def wq(kc):
            return wqt[:, 768 * kc:768 * (kc + 1)]

        # vt tiles [128, 512]: per head h: cols 64h:64h+32 = v_h, rest 1.0.
        vt = []
        for jc in range(8):
            tv = singles.tile([128, 512], bf16, tag=f"vt_{jc}")
            nc.gpsimd.memset(tv, 1.0)
            vt.append(tv)

        # out_allT: packed final-GEMM rhs, 2 tiles [128, 1024] bf16.
        # tile Q rows 32*hq + d = head 4Q+hq; all 128 rows written by norm.
        out_allT = []
        for t in range(2):
            ta = singles.tile([128, N], bf16, tag=f"oa_{t}")
            out_allT.append(ta)

        qb = [None, None]
        kb = [None, None]
        for kind in ("q", "k"):
            for i in range(2):
                dst = singles.tile([128, N], bf16, tag=f"{kind}b_{i}",
                                   name=f"{kind}b{i}")
                (qb if kind == "q" else kb)[i] = dst

        def emit_qkv_chunk(kind, i, nh):
            # kind 'q' -> wq cols 128i, 'k' -> 256+128i
            col0 = (0 if kind == "q" else 256) + 128 * i
            dst = (qb if kind == "q" else kb)[i]
            ps = acc_psum.tile([128, 512], f32, tag="acc",
                               name=f"qkv_{kind}{i}_{nh}")
            for kc in range(2):
                nc.tensor.matmul(
                    ps,
                    wq(kc)[:, col0:col0 + 128],
                    xb(kc)[:, nh * 512:(nh + 1) * 512],
                    start=(kc == 0),
                    stop=(kc == 1),
                )
            nc.vector.tensor_copy(out=dst[:, nh * 512:(nh + 1) * 512], in_=ps)

        def emit_vt(jc):
            ps = acc_psum.tile([128, 256], f32, tag="acc", name=f"vt_{jc}")
            for kc in range(2):
                nc.tensor.matmul(
                    ps,
                    xb(kc)[:, jc * 128:(jc + 1) * 128],
                    wq(kc)[:, 512:768],
                    start=(kc == 0),
                    stop=(kc == 1),
                )
            vb = vt[jc][:, 0:1]
            out_ap = AP(vb.tensor, vb.offset, [list(vb.ap[0]), [64, 8], [1, 32]])
            pb = ps[:, 0:1]
            in_ap = AP(pb.tensor, pb.offset, [list(pb.ap[0]), [32, 8], [1, 32]])
            nc.vector.tensor_copy(out=out_ap, in_=in_ap)

        # ---- sim ring machinery ----
        # groups g=0..3: ih=g//2, Q=g%2. unit w=0..31: pq=w//16,
        # jc=(w%16)//2, which=w%2, hq=2pq+which. In the LAST group pairB
        # units go first (pq = 1 - w//16) so its AV can run early and only
        # pairA's tail waits on the final exp.
        # global unit u = 32g + w; ring = u//3, slot = u%3.
        exp_slices = {}
        state = {"psum": None, "exp": None, "units": 0, "u": 0}

        def flush_ring():
            if state["psum"] is None:
                return
            w = state["units"] * 512
            nc.scalar.activation(
                out=state["exp"][:, 0:w],
                in_=state["psum"][:, 0:w],
                func=mybir.ActivationFunctionType.Exp,
            )
            state["psum"] = None
            state["exp"] = None
            state["units"] = 0

        def unit_pq(g, w):
            return (1 - w // 16) if g == 3 else (w // 16)

        def emit_sim_units(g, w_lo, w_hi, last=False):
            ih, Q = g // 2, g % 2
            for w in range(w_lo, w_hi):
                pq, jc, which = unit_pq(g, w), (w % 16) // 2, w % 2
                hq = 2 * pq + which
                u = state["u"]
                state["u"] += 1
                if state["psum"] is None:
                    state["psum"] = sim_psum.tile([128, 1536], f32, tag="sim",
                                                  name=f"sim_{u}")
                    state["exp"] = exp_pool.tile([128, 1536], bf16, tag="exp",
                                                 name=f"exp_{u}")
                s = state["units"]
                tp = (96, 0) if hq == 3 else None
                nc.tensor.matmul(
                    state["psum"][:, s * 512:(s + 1) * 512],
                    kb[Q][32 * hq:32 * (hq + 1), jc * 128:(jc + 1) * 128],
                    qb[Q][32 * hq:32 * (hq + 1), ih * 512:(ih + 1) * 512],
                    start=True,
                    stop=True,
                    tile_position=tp,
                )
                exp_slices[(g, pq, jc, which)] = (state["exp"], s)
                state["units"] += 1
                if state["units"] == 3:
                    flush_ring()
            if last:
                flush_ring()

        # ---- AV + normalize ----
        def av_pair(g, pq):
            ih, Q = g // 2, g % 2
            hA, hB = 4 * Q + 2 * pq, 4 * Q + 2 * pq + 1  # global heads
            ps = acc_psum.tile([128, 512], f32, tag="acc", name=f"av_{g}_{pq}")
            for jc in range(8):
                eA, sA = exp_slices[(g, pq, jc, 0)]
                eB, sB = exp_slices[(g, pq, jc, 1)]
                st, sp = (jc == 0), (jc == 7)
                nc.tensor.matmul(
                    ps[0:64, :], vt[jc][:, 64 * hA:64 * hA + 64],
                    eA[:, sA * 512:(sA + 1) * 512], start=st, stop=sp)
                nc.tensor.matmul(
                    ps[64:128, :], vt[jc][:, 64 * hB:64 * hB + 64],
                    eB[:, sB * 512:(sB + 1) * 512], start=st, stop=sp,
                    tile_position=(0, 64))
            # rows 0:32 mainA, 32:64 denA, 64:96 mainB, 96:128 denB
            rc = rc_pool.tile([128, 512], f32, tag="rc")
            nc.vector.reciprocal_approx_fast(out=rc[:, :], in_=ps[:, :])
            dst = out_allT[Q]
            r0 = 64 * pq
            nc.vector.tensor_mul(
                out=dst[r0:r0 + 32, ih * 512:(ih + 1) * 512],
                in0=ps[0:32, :], in1=rc[32:64, :])
            nc.vector.tensor_mul(
                out=dst[r0 + 32:r0 + 64, ih * 512:(ih + 1) * 512],
                in0=ps[64:96, :], in1=rc[96:128, :])

        def emit_proj(nh, oc, dma_engine):
            ps = acc_psum.tile([128, 512], f32, tag="acc",
                               name=f"proj_{nh}_{oc}")
            for t in range(2):
                nc.tensor.matmul(
                    ps,
                    wot[:, 256 * t + 128 * oc:256 * t + 128 * oc + 128],
                    out_allT[t][:, nh * 512:(nh + 1) * 512],
                    start=(t == 0),
                    stop=(t == 1),
                )
            ys = evac.tile([128, 512], bf16, tag="y")
            nc.vector.tensor_scalar_add(out=ys, in0=ps,
                                        scalar1=bias2[:, oc:oc + 1])
            dma_engine.dma_start(
                out=y_ap[oc * 128:(oc + 1) * 128, nh * 512:(nh + 1) * 512],
                in_=ys,
            )

        # ---- emission schedule (block-contiguous, half-group granularity) ----
        emit_qkv_chunk("k", 0, 0)
        emit_qkv_chunk("q", 0, 0)
        emit_qkv_chunk("k", 0, 1)
        emit_qkv_chunk("q", 0, 1)
        emit_sim_units(0, 0, 16)
        emit_qkv_chunk("k", 1, 0)
        emit_qkv_chunk("k", 1, 1)
        emit_qkv_chunk("q", 1, 0)
        emit_qkv_chunk("q", 1, 1)
        emit_sim_units(0, 16, 32)
        for jc in range(8):
            emit_vt(jc)
        emit_sim_units(1, 0, 16)
        av_pair(0, 0)
        emit_sim_units(1, 16, 32)
        av_pair(0, 1)
        emit_sim_units(2, 0, 16)
        av_pair(1, 0)
        emit_sim_units(2, 16, 32)
        av_pair(1, 1)
        # proj of token half 0: out_allT[*][:, 0:512] complete (groups 0+1)
        emit_proj(0, 0, nc.sync)
        emit_sim_units(3, 0, 16)       # pairB units first (unit_pq flip)
        av_pair(2, 0)
        emit_proj(0, 1, nc.sync)
        emit_sim_units(3, 16, 24)
        av_pair(2, 1)
        emit_sim_units(3, 24, 32, last=True)
        av_pair(3, 1)                  # units were w 0..15: exp'd early
        av_pair(3, 0)                  # tail: waits the final rings
        emit_proj(1, 0, nc.scalar)
        emit_proj(1, 1, nc.sync)

        if dbg is not None:
            for nm, tile_ in (("qb0", qb[0]), ("kb0", kb[0]), ("vt0", vt[0]),
                              ("oa0", out_allT[0]), ("oa1", out_allT[1])):
                if nm in dbg:
                    nc.sync.dma_start(out=dbg[nm], in_=tile_)
            if "exp0" in dbg:
                et, s = exp_slices[(0, 0)]
                nc.sync.dma_start(out=dbg["exp0"], in_=et[:, s * 512:(s + 1) * 512])


def _prep_weights(w_qkv, w_out, b_out):
    """Host-side weight preparation (numpy)."""
    wq = w_qkv.astype(np.float32).copy()
    wq[0:DIM_HEAD] *= SCALE                      # fold softmax scale into w_q
    wqkvT = np.ascontiguousarray(wq.T)                        # [256, 768]
    wqkvT = np.concatenate([wqkvT[0:128], wqkvT[128:256]], axis=1)  # [128,1536]
    wqkvT = np.ascontiguousarray(wqkvT).astype(_BF16)
    woutT = np.ascontiguousarray(w_out.astype(np.float32).T)  # [256, 256]
    woutT = np.ascontiguousarray(
        np.concatenate([woutT[0:128], woutT[128:256]], axis=1)).astype(_BF16)
    bout = np.ascontiguousarray(
        b_out.astype(np.float32).reshape(2, 128).T)           # [128, 2]
    return wqkvT, woutT, bout


def _build_program():
    global _PROGRAM
    if _PROGRAM is not None:
        return _PROGRAM
    import concourse.tile as tile
    from concourse import bacc, mybir

    nc = bacc.Bacc("TRN2", target_bir_lowering=False, debug=False,
                   num_devices=NCORES)
    x_ap = nc.dram_tensor("x", [128, 2048], mybir.dt.bfloat16,
                          kind="ExternalInput").ap()
    wqkvT_ap = nc.dram_tensor("wqkvT", [128, 1536], mybir.dt.bfloat16,
                              kind="ExternalInput").ap()
    woutT_ap = nc.dram_tensor("woutT", [128, 512], mybir.dt.bfloat16,
                              kind="ExternalInput").ap()
    bout_ap = nc.dram_tensor("bout", [128, 2], mybir.dt.float32,
                             kind="ExternalInput").ap()
    y_ap = nc.dram_tensor("y", [DIM, N], mybir.dt.bfloat16,
                          kind="ExternalOutput").ap()
    with tile.TileContext(nc) as tc:
        build_kernel_body(tc, y_ap, x_ap, wqkvT_ap, woutT_ap, bout_ap)
    nc.compile()
    _PROGRAM = nc
    return nc


def kernel(x, w_qkv, w_out, b_out, trace=False):
    """Full-input entry point: shard over batch, run on 8 cores, gather."""
    from concourse import bass_utils

    nc = _build_program()
    wqkvT, woutT, bout = _prep_weights(w_qkv, w_out, b_out)
    in_maps = []
    for b in range(B):
        xb = np.asarray(x[b], dtype=np.float32).reshape(DIM, N)
        xb = np.ascontiguousarray(
            np.concatenate([xb[0:128], xb[128:256]], axis=1)).astype(_BF16)
        in_maps.append({
            "x": xb,
            "wqkvT": wqkvT,
            "woutT": woutT,
            "bout": bout,
        })
    res = bass_utils.run_bass_kernel_spmd(
        nc, in_maps, core_ids=list(range(NCORES)), trace=trace)
    y = np.stack([
        res.results[b]["y"].astype(np.float32).reshape(DIM, H, W)
        for b in range(B)
    ])
    kernel.last_results = res
    return y


# revision 11
# speedup vs baseline: 1.2706x; 1.2040x over previous
"""Trainium2 Bass kernel for nn_Attention2D (B=8, C=256, H=W=32, 8 heads, d=32).

Strategy: data-parallel over batch, one batch element per NeuronCore (8 cores).

Per-core pipeline (n = H*W = 1024 tokens, head dim d = 32):
  load:   host-packed single-DMA inputs (x pre-cast bf16), issued from
          sync/scalar/gpsimd engines in parallel.
  qkv:    q = (scale*w_q) @ x, k = w_k @ x   ([256,1024] head-major, bf16)
          vT[jc] = x[:, jc]^T @ w_v^T, evacuated via strided cast into
          [v_h(32) | ones(32)] x 8 layout ([128,512] bf16 per j-chunk).
  sim^T:  per (head, j-chunk): matmul(lhsT=k slice [32,128], rhs=q slice
          [32,512]) -> PSUM ring tiles [128,1536] (3 units); 4 heads share
          the PE array via K=32 row groups. Softmax max-subtraction skipped
          (logits ~N(0,0.8), max |sim| ~ 4.8; exp safe in fp32).
  exp:    ACT engine Exp over ring tiles -> bf16 SBUF. ACT is the kernel
          roofline: 8.4M exps/core at 128 lanes / 1.2 GHz + per-inst ovh.
  AV:     per (group, pair): accumulate over j-chunks into one PSUM tile:
            headA: lhsT = vt[:, 64hA:64hA+64] ([v|ones], M=64) -> rows 0:64
                   (rows 0:32 main, 32:64 denominator replicated)
            headB: same at tile_position (0,64) -> rows 64:128
          The ones columns make the softmax denominator free on PE.
  norm:   rc = reciprocal_approx_fast(psum); out = main * rc via
          cross-partition-offset tensor_mul into packed out_allT tiles.
  proj:   incremental per output half: y = w_outT^T @ out_allT + b_out,
          bf16 output.

ACT (exp) is the pacing engine. PE work other than sims (qkv, vT, AV of the
previous group, projection) is emitted in half-group blocks between sim
blocks: the 2-deep PSUM ring keeps ACT busy across each ~3.4us AV block, and
block-contiguous matmuls of one shape avoid PE weight-config thrash (a
per-ring interleave measured ~2x slower matmuls).
"""

import numpy as np
import ml_dtypes
from collections import deque

B, DIM, H, W = 8, 256, 32, 32
NUM_HEADS = 8
DIM_HEAD = 256
D = DIM_HEAD // NUM_HEADS          # 32 per-head dim
N = H * W                          # 1024 tokens
SCALE = (DIM_HEAD / NUM_HEADS) ** (-0.5)
NCORES = 8

_BF16 = ml_dtypes.bfloat16

_PROGRAM = None  # compiled Bass program cache (one per process)


def build_kernel_body(tc, y_ap, x_ap, wqkvT_ap, woutT_ap, bout_ap, dbg=None):
    """Emit the per-core attention program into TileContext tc.

    DRAM tensors (host-packed, 128-partition layout):
      x_ap:     [128, 2048] bf16  (cols 1024*kc+i = x[128*kc + c, i])
      wqkvT_ap: [128, 1536] bf16  (cols 768*kc+o = w_qkvT[128*kc + c, o];
                                   o: 0:256 q pre-scaled, 256:512 k, 512:768 v)
      woutT_ap: [128, 512]  bf16  (cols 256*t+o = w_outT[128*t + hd, o])
      bout_ap:  [128, 2]    fp32  (col oc = b_out[128*oc + c])
      y_ap:     [256, 1024] bf16 out
    """
    from contextlib import ExitStack
    from concourse import mybir
    from concourse.ap import AP

    nc = tc.nc
    f32 = mybir.dt.float32
    bf16 = mybir.dt.bfloat16

    with ExitStack() as ctx:
        singles = ctx.enter_context(tc.tile_pool(name="singles", bufs=1))
        evac = ctx.enter_context(tc.tile_pool(name="evac", bufs=2))
        exp_pool = ctx.enter_context(tc.tile_pool(name="exp", bufs=24))
        rc_pool = ctx.enter_context(tc.tile_pool(name="rc", bufs=2))
        sim_psum = ctx.enter_context(tc.tile_pool(name="simp", bufs=2, space="PSUM"))
        acc_psum = ctx.enter_context(tc.tile_pool(name="accp", bufs=2, space="PSUM"))

        # ---- phase 0: input DMAs, split across sync+scalar issue engines
        # (two HWDGE rings -> parallel transfer queues) ----
        xt = singles.tile([128, 2048], bf16, tag="xt")
        nc.sync.dma_start(out=xt[:, 0:1024], in_=x_ap[:, 0:1024])
        wqt = singles.tile([128, 1536], bf16, tag="wqt")
        nc.scalar.dma_start(out=xt[:, 1024:2048], in_=x_ap[:, 1024:2048])
        nc.sync.dma_start(out=wqt[:, 0:768], in_=wqkvT_ap[:, 0:768])
        nc.scalar.dma_start(out=wqt[:, 768:1536], in_=wqkvT_ap[:, 768:1536])
        wot = singles.tile([128, 512], bf16, tag="wot")
        nc.sync.dma_start(out=wot, in_=woutT_ap)
        bias2 = singles.tile([128, 2], f32, tag="bias2")
        nc.sync.dma_start(out=bias2, in_=bout_ap)

        def xb(kc):
            return xt[:, 1024 * kc:1024 * (kc + 1)]

        def wq(kc):
            return wqt[:, 768 * kc:768 * (kc + 1)]

        # vt tiles [128, 512]: per head h: cols 64h:64h+32 = v_h, rest 1.0.
        vt = []
        for jc in range(8):
            tv = singles.tile([128, 512], bf16, tag=f"vt_{jc}")
            nc.gpsimd.memset(tv, 1.0)
            vt.append(tv)

        # out_allT: packed final-GEMM rhs, 2 tiles [128, 1024] bf16.
        # tile Q rows 32*hq + d = head 4Q+hq; all 128 rows written by norm.
        out_allT = []
        for t in range(2):
            ta = singles.tile([128, N], bf16, tag=f"oa_{t}")
            out_allT.append(ta)

        qb = [None, None]
        kb = [None, None]
        for kind in ("q", "k"):
            for i in range(2):
                dst = singles.tile([128, N], bf16, tag=f"{kind}b_{i}",
                                   name=f"{kind}b{i}")
                (qb if kind == "q" else kb)[i] = dst

        def emit_qkv_chunk(kind, i, nh):
            # kind 'q' -> wq cols 128i, 'k' -> 256+128i
            col0 = (0 if kind == "q" else 256) + 128 * i
            dst = (qb if kind == "q" else kb)[i]
            ps = acc_psum.tile([128, 512], f32, tag="acc",
                               name=f"qkv_{kind}{i}_{nh}")
            for kc in range(2):
                nc.tensor.matmul(
                    ps,
                    wq(kc)[:, col0:col0 + 128],
                    xb(kc)[:, nh * 512:(nh + 1) * 512],
                    start=(kc == 0),
                    stop=(kc == 1),
                )
            nc.vector.tensor_copy(out=dst[:, nh * 512:(nh + 1) * 512], in_=ps)

        def emit_vt(jc):
            ps = acc_psum.tile([128, 256], f32, tag="acc", name=f"vt_{jc}")
            for kc in range(2):
                nc.tensor.matmul(
                    ps,
                    xb(kc)[:, jc * 128:(jc + 1) * 128],
                    wq(kc)[:, 512:768],
                    start=(kc == 0),
                    stop=(kc == 1),
                )
            vb = vt[jc][:, 0:1]
            out_ap = AP(vb.tensor, vb.offset, [list(vb.ap[0]), [64, 8], [1, 32]])
            pb = ps[:, 0:1]
            in_ap = AP(pb.tensor, pb.offset, [list(pb.ap[0]), [32, 8], [1, 32]])
            nc.vector.tensor_copy(out=out_ap, in_=in_ap)

        # ---- sim ring machinery ----
        # groups g=0..3: ih=g//2, Q=g%2. unit w=0..31: pq=w//16,
        # jc=(w%16)//2, which=w%2, hq=2pq+which. In the LAST group pairB
        # units go first (pq = 1 - w//16) so its AV can run early and only
        # pairA's tail waits on the final exp.
        # global unit u = 32g + w; ring = u//3, slot = u%3.
        exp_slices = {}
        state = {"psum": None, "exp": None, "units": 0, "u": 0}

        def flush_ring():
            if state["psum"] is None:
                return
            w = state["units"] * 512
            nc.scalar.activation(
                out=state["exp"][:, 0:w],
                in_=state["psum"][:, 0:w],
                func=mybir.ActivationFunctionType.Exp,
            )
            state["psum"] = None
            state["exp"] = None
            state["units"] = 0

        def unit_pq(g, w):
            return (1 - w // 16) if g == 3 else (w // 16)

        def emit_sim_units(g, w_lo, w_hi, last=False):
            ih, Q = g // 2, g % 2
            for w in range(w_lo, w_hi):
                pq, jc, which = unit_pq(g, w), (w % 16) // 2, w % 2
                hq = 2 * pq + which
                u = state["u"]
                state["u"] += 1
                if state["psum"] is None:
                    state["psum"] = sim_psum.tile([128, 1536], f32, tag="sim",
                                                  name=f"sim_{u}")
                    state["exp"] = exp_pool.tile([128, 1536], bf16, tag="exp",
                                                 name=f"exp_{u}")
                s = state["units"]
                tp = (96, 0) if hq == 3 else None
                nc.tensor.matmul(
                    state["psum"][:, s * 512:(s + 1) * 512],
                    kb[Q][32 * hq:32 * (hq + 1), jc * 128:(jc + 1) * 128],
                    qb[Q][32 * hq:32 * (hq + 1), ih * 512:(ih + 1) * 512],
                    start=True,
                    stop=True,
                    tile_position=tp,
                )
                exp_slices[(g, pq, jc, which)] = (state["exp"], s)
                state["units"] += 1
                if state["units"] == 3:
                    flush_ring()
            if last:
                flush_ring()

        # ---- AV + normalize ----
        def av_pair(g, pq):
            ih, Q = g // 2, g % 2
            hA, hB = 4 * Q + 2 * pq, 4 * Q + 2 * pq + 1  # global heads
            ps = acc_psum.tile([128, 512], f32, tag="acc", name=f"av_{g}_{pq}")
            for jc in range(8):
                eA, sA = exp_slices[(g, pq, jc, 0)]
                eB, sB = exp_slices[(g, pq, jc, 1)]
                st, sp = (jc == 0), (jc == 7)
                nc.tensor.matmul(
                    ps[0:64, :], vt[jc][:, 64 * hA:64 * hA + 64],
                    eA[:, sA * 512:(sA + 1) * 512], start=st, stop=sp)
                nc.tensor.matmul(
                    ps[64:128, :], vt[jc][:, 64 * hB:64 * hB + 64],
                    eB[:, sB * 512:(sB + 1) * 512], start=st, stop=sp,
                    tile_position=(0, 64))
            # rows 0:32 mainA, 32:64 denA, 64:96 mainB, 96:128 denB
            rc = rc_pool.tile([128, 512], f32, tag="rc")
            nc.vector.reciprocal_approx_fast(out=rc[:, :], in_=ps[:, :])
            dst = out_allT[Q]
            r0 = 64 * pq
            nc.vector.tensor_mul(
                out=dst[r0:r0 + 32, ih * 512:(ih + 1) * 512],
                in0=ps[0:32, :], in1=rc[32:64, :])
            nc.vector.tensor_mul(
                out=dst[r0 + 32:r0 + 64, ih * 512:(ih + 1) * 512],
                in0=ps[64:96, :], in1=rc[96:128, :])

        def emit_proj(nh, oc, dma_engine, bias_on_act=False):
            ps = acc_psum.tile([128, 512], f32, tag="acc",
                               name=f"proj_{nh}_{oc}")
            for t in range(2):
                nc.tensor.matmul(
                    ps,
                    wot[:, 256 * t + 128 * oc:256 * t + 128 * oc + 128],
                    out_allT[t][:, nh * 512:(nh + 1) * 512],
                    start=(t == 0),
                    stop=(t == 1),
                )
            ys = evac.tile([128, 512], bf16, tag="y")
            if bias_on_act:
                # tail only: ACT is idle once the last exp has issued
                nc.scalar.add(out=ys, in_=ps, add=bias2[:, oc:oc + 1])
            else:
                nc.vector.tensor_scalar_add(out=ys, in0=ps,
                                            scalar1=bias2[:, oc:oc + 1])
            dma_engine.dma_start(
                out=y_ap[oc * 128:(oc + 1) * 128, nh * 512:(nh + 1) * 512],
                in_=ys,
            )

        # ---- emission schedule (block-contiguous, half-group granularity) ----
        emit_qkv_chunk("k", 0, 0)
        emit_qkv_chunk("q", 0, 0)
        emit_qkv_chunk("k", 0, 1)
        emit_qkv_chunk("q", 0, 1)
        emit_sim_units(0, 0, 16)
        emit_qkv_chunk("k", 1, 0)
        emit_qkv_chunk("k", 1, 1)
        emit_qkv_chunk("q", 1, 0)
        emit_qkv_chunk("q", 1, 1)
        emit_sim_units(0, 16, 32)
        for jc in range(8):
            emit_vt(jc)
        emit_sim_units(1, 0, 32)
        av_pair(0, 0)
        av_pair(0, 1)
        emit_sim_units(2, 0, 32)
        av_pair(1, 0)
        av_pair(1, 1)
        # proj of token half 0: out_allT[*][:, 0:512] complete (groups 0+1)
        emit_proj(0, 0, nc.sync)
        emit_proj(0, 1, nc.sync)
        emit_sim_units(3, 0, 16)       # pairB units first (unit_pq flip)
        av_pair(2, 0)
        emit_sim_units(3, 16, 24)
        av_pair(2, 1)
        emit_sim_units(3, 24, 32, last=True)
        av_pair(3, 1)                  # units were w 0..15: exp'd early
        av_pair(3, 0)                  # tail: waits the final rings
        emit_proj(1, 0, nc.scalar, bias_on_act=True)
        emit_proj(1, 1, nc.sync)

        if dbg is not None:
            for nm, tile_ in (("qb0", qb[0]), ("kb0", kb[0]), ("vt0", vt[0]),
                              ("oa0", out_allT[0]), ("oa1", out_allT[1])):
                if nm in dbg:
                    nc.sync.dma_start(out=dbg[nm], in_=tile_)
            if "exp0" in dbg:
                et, s = exp_slices[(0, 0)]
                nc.sync.dma_start(out=dbg["exp0"], in_=et[:, s * 512:(s + 1) * 512])


def _prep_weights(w_qkv, w_out, b_out):
    """Host-side weight preparation (numpy)."""
    wq = w_qkv.astype(np.float32).copy()
    wq[0:DIM_HEAD] *= SCALE                      # fold softmax scale into w_q
    wqkvT = np.ascontiguousarray(wq.T)                        # [256, 768]
    wqkvT = np.concatenate([wqkvT[0:128], wqkvT[128:256]], axis=1)  # [128,1536]
    wqkvT = np.ascontiguousarray(wqkvT).astype(_BF16)
    woutT = np.ascontiguousarray(w_out.astype(np.float32).T)  # [256, 256]
    woutT = np.ascontiguousarray(
        np.concatenate([woutT[0:128], woutT[128:256]], axis=1)).astype(_BF16)
    bout = np.ascontiguousarray(
        b_out.astype(np.float32).reshape(2, 128).T)           # [128, 2]
    return wqkvT, woutT, bout


def _build_program():
    global _PROGRAM
    if _PROGRAM is not None:
        return _PROGRAM
    import concourse.tile as tile
    from concourse import bacc, mybir

    nc = bacc.Bacc("TRN2", target_bir_lowering=False, debug=False,
                   num_devices=NCORES)
    x_ap = nc.dram_tensor("x", [128, 2048], mybir.dt.bfloat16,
                          kind="ExternalInput").ap()
    wqkvT_ap = nc.dram_tensor("wqkvT", [128, 1536], mybir.dt.bfloat16,
                              kind="ExternalInput").ap()
    woutT_ap = nc.dram_tensor("woutT", [128, 512], mybir.dt.bfloat16,
                              kind="ExternalInput").ap()
    bout_ap = nc.dram_tensor("bout", [128, 2], mybir.dt.float32,
                             kind="ExternalInput").ap()
    y_ap = nc.dram_tensor("y", [DIM, N], mybir.dt.bfloat16,
                          kind="ExternalOutput").ap()
    with tile.TileContext(nc) as tc:
        build_kernel_body(tc, y_ap, x_ap, wqkvT_ap, woutT_ap, bout_ap)
    nc.compile()
    _PROGRAM = nc
    return nc


def kernel(x, w_qkv, w_out, b_out, trace=False):
    """Full-input entry point: shard over batch, run on 8 cores, gather."""
    from concourse import bass_utils

    nc = _build_program()
    wqkvT, woutT, bout = _prep_weights(w_qkv, w_out, b_out)
    in_maps = []
    for b in range(B):
        xb = np.asarray(x[b], dtype=np.float32).reshape(DIM, N)
        xb = np.ascontiguousarray(
            np.concatenate([xb[0:128], xb[128:256]], axis=1)).astype(_BF16)
        in_maps.append({
            "x": xb,
            "wqkvT": wqkvT,
            "woutT": woutT,
            "bout": bout,
        })
    res = bass_utils.run_bass_kernel_spmd(
        nc, in_maps, core_ids=list(range(NCORES)), trace=trace)
    y = np.stack([
        res.results[b]["y"].astype(np.float32).reshape(DIM, H, W)
        for b in range(B)
    ])
    kernel.last_results = res
    return y


# revision 16
# speedup vs baseline: 1.2784x; 1.0061x over previous
"""Trainium2 Bass kernel for nn_Attention2D (B=8, C=256, H=W=32, 8 heads, d=32).

Strategy: data-parallel over batch, one batch element per NeuronCore (8 cores).

Per-core pipeline (n = H*W = 1024 tokens, head dim d = 32):
  load:   host-packed single-DMA inputs (x pre-cast bf16), issued from
          sync/scalar/gpsimd engines in parallel.
  qkv:    q = (scale*w_q) @ x, k = w_k @ x   ([256,1024] head-major, bf16)
          vT[jc] = x[:, jc]^T @ w_v^T, evacuated via strided cast into
          [v_h(32) | ones(32)] x 8 layout ([128,512] bf16 per j-chunk).
  sim^T:  per (head, j-chunk): matmul(lhsT=k slice [32,128], rhs=q slice
          [32,512]) -> PSUM ring tiles [128,1536] (3 units); 4 heads share
          the PE array via K=32 row groups. Softmax max-subtraction skipped
          (logits ~N(0,0.8), max |sim| ~ 4.8; exp safe in fp32).
  exp:    ACT engine Exp over ring tiles -> bf16 SBUF. ACT is the kernel
          roofline: 8.4M exps/core at 128 lanes / 1.2 GHz + per-inst ovh.
  AV:     per (group, pair): accumulate over j-chunks into one PSUM tile:
            headA: lhsT = vt[:, 64hA:64hA+64] ([v|ones], M=64) -> rows 0:64
                   (rows 0:32 main, 32:64 denominator replicated)
            headB: same at tile_position (0,64) -> rows 64:128
          The ones columns make the softmax denominator free on PE.
  norm:   rc = reciprocal_approx_fast(psum); out = main * rc via
          cross-partition-offset tensor_mul into packed out_allT tiles.
  proj:   incremental per output half: y = w_outT^T @ out_allT + b_out,
          bf16 output.

ACT (exp) is the pacing engine. PE work other than sims (qkv, vT, AV of the
previous group, projection) is emitted in half-group blocks between sim
blocks: the 2-deep PSUM ring keeps ACT busy across each ~3.4us AV block, and
block-contiguous matmuls of one shape avoid PE weight-config thrash (a
per-ring interleave measured ~2x slower matmuls).
"""

import numpy as np
import ml_dtypes
from collections import deque

B, DIM, H, W = 8, 256, 32, 32
NUM_HEADS = 8
DIM_HEAD = 256
D = DIM_HEAD // NUM_HEADS          # 32 per-head dim
N = H * W                          # 1024 tokens
SCALE = (DIM_HEAD / NUM_HEADS) ** (-0.5)
NCORES = 8

_BF16 = ml_dtypes.bfloat16

_PROGRAM = None  # compiled Bass program cache (one per process)


def build_kernel_body(tc, y_ap, x_ap, wqkvT_ap, woutT_ap, bout_ap, dbg=None):
    """Emit the per-core attention program into TileContext tc.

    DRAM tensors (host-packed, 128-partition layout):
      x_ap:     [128, 2048] bf16  (cols 1024*kc+i = x[128*kc + c, i])
      wqkvT_ap: [128, 1536] bf16  (cols 768*kc+o = w_qkvT[128*kc + c, o];
                                   o: 0:256 q pre-scaled, 256:512 k, 512:768 v)
      woutT_ap: [128, 512]  bf16  (cols 256*t+o = w_outT[128*t + hd, o])
      bout_ap:  [128, 2]    fp32  (col oc = b_out[128*oc + c])
      y_ap:     [256, 1024] bf16 out
    """
    from contextlib import ExitStack
    from concourse import mybir
    from concourse.ap import AP

    nc = tc.nc
    f32 = mybir.dt.float32
    bf16 = mybir.dt.bfloat16

    with ExitStack() as ctx:
        singles = ctx.enter_context(tc.tile_pool(name="singles", bufs=1))
        evac = ctx.enter_context(tc.tile_pool(name="evac", bufs=2))
        exp_pool = ctx.enter_context(tc.tile_pool(name="exp", bufs=24))
        rc_pool = ctx.enter_context(tc.tile_pool(name="rc", bufs=2))
        sim_psum = ctx.enter_context(tc.tile_pool(name="simp", bufs=2, space="PSUM"))
        acc_psum = ctx.enter_context(tc.tile_pool(name="accp", bufs=2, space="PSUM"))

        # ---- phase 0: input DMAs, split across sync+scalar issue engines
        # (two HWDGE rings -> parallel transfer queues), dependency-ordered:
        # the first qkv matmuls need x nh0-halves and the k weight columns,
        # so those chunks are issued first on each engine.
        xt = singles.tile([128, 2048], bf16, tag="xt")
        wqt = singles.tile([128, 1536], bf16, tag="wqt")
        wot = singles.tile([128, 512], bf16, tag="wot")
        bias2 = singles.tile([128, 2], f32, tag="bias2")
        # wq halves laid out [k(256) | q(256) | v(256)] per kc on host
        def in_chunks(eng, x0, w0):
            eng.dma_start(out=xt[:, x0:x0 + 512], in_=x_ap[:, x0:x0 + 512])
            eng.dma_start(out=wqt[:, w0:w0 + 256], in_=wqkvT_ap[:, w0:w0 + 256])
            eng.dma_start(out=wqt[:, w0 + 256:w0 + 512],
                          in_=wqkvT_ap[:, w0 + 256:w0 + 512])
            eng.dma_start(out=xt[:, x0 + 512:x0 + 1024],
                          in_=x_ap[:, x0 + 512:x0 + 1024])
            eng.dma_start(out=wqt[:, w0 + 512:w0 + 768],
                          in_=wqkvT_ap[:, w0 + 512:w0 + 768])

        in_chunks(nc.sync, 0, 0)
        in_chunks(nc.scalar, 1024, 768)
        nc.sync.dma_start(out=wot, in_=woutT_ap)
        nc.sync.dma_start(out=bias2, in_=bout_ap)

        def xb(kc):
            return xt[:, 1024 * kc:1024 * (kc + 1)]

        def wq(kc):
            return wqt[:, 768 * kc:768 * (kc + 1)]

        # vt tiles [128, 512]: per head h: cols 64h:64h+32 = v_h, rest 1.0.
        vt = []
        for jc in range(8):
            tv = singles.tile([128, 512], bf16, tag=f"vt_{jc}")
            nc.gpsimd.memset(tv, 1.0)
            vt.append(tv)

        # out_allT: packed final-GEMM rhs, 2 tiles [128, 1024] bf16.
        # tile Q rows 32*hq + d = head 4Q+hq; all 128 rows written by norm.
        out_allT = []
        for t in range(2):
            ta = singles.tile([128, N], bf16, tag=f"oa_{t}")
            out_allT.append(ta)

        qb = [None, None]
        kb = [None, None]
        for kind in ("q", "k"):
            for i in range(2):
                dst = singles.tile([128, N], bf16, tag=f"{kind}b_{i}",
                                   name=f"{kind}b{i}")
                (qb if kind == "q" else kb)[i] = dst

        def emit_qkv_chunk(kind, i, nh):
            # wq half layout [k|q|v]: 'k' -> cols 128i, 'q' -> 256+128i
            col0 = (256 if kind == "q" else 0) + 128 * i
            dst = (qb if kind == "q" else kb)[i]
            ps = acc_psum.tile([128, 512], f32, tag="acc",
                               name=f"qkv_{kind}{i}_{nh}")
            for kc in range(2):
                nc.tensor.matmul(
                    ps,
                    wq(kc)[:, col0:col0 + 128],
                    xb(kc)[:, nh * 512:(nh + 1) * 512],
                    start=(kc == 0),
                    stop=(kc == 1),
                )
            nc.vector.tensor_copy(out=dst[:, nh * 512:(nh + 1) * 512], in_=ps)

        def emit_vt(jc):
            ps = acc_psum.tile([128, 256], f32, tag="acc", name=f"vt_{jc}")
            for kc in range(2):
                nc.tensor.matmul(
                    ps,
                    xb(kc)[:, jc * 128:(jc + 1) * 128],
                    wq(kc)[:, 512:768],
                    start=(kc == 0),
                    stop=(kc == 1),
                )
            vb = vt[jc][:, 0:1]
            out_ap = AP(vb.tensor, vb.offset, [list(vb.ap[0]), [64, 8], [1, 32]])
            pb = ps[:, 0:1]
            in_ap = AP(pb.tensor, pb.offset, [list(pb.ap[0]), [32, 8], [1, 32]])
            nc.vector.tensor_copy(out=out_ap, in_=in_ap)

        # ---- sim ring machinery ----
        # groups g=0..3: ih=g//2, Q=g%2. unit w=0..31: pq=w//16,
        # jc=(w%16)//2, which=w%2, hq=2pq+which. In the LAST group pairB
        # units go first (pq = 1 - w//16) so its AV can run early and only
        # pairA's tail waits on the final exp.
        # global unit u = 32g + w; ring = u//3, slot = u%3.
        exp_slices = {}
        state = {"psum": None, "exp": None, "units": 0, "u": 0}

        def flush_ring():
            if state["psum"] is None:
                return
            w = state["units"] * 512
            nc.scalar.activation(
                out=state["exp"][:, 0:w],
                in_=state["psum"][:, 0:w],
                func=mybir.ActivationFunctionType.Exp,
            )
            state["psum"] = None
            state["exp"] = None
            state["units"] = 0

        def unit_pq(g, w):
            return (1 - w // 16) if g == 3 else (w // 16)

        def emit_sim_units(g, w_lo, w_hi, last=False):
            ih, Q = g // 2, g % 2
            for w in range(w_lo, w_hi):
                pq, jc, which = unit_pq(g, w), (w % 16) // 2, w % 2
                hq = 2 * pq + which
                u = state["u"]
                state["u"] += 1
                if state["psum"] is None:
                    state["psum"] = sim_psum.tile([128, 1536], f32, tag="sim",
                                                  name=f"sim_{u}")
                    state["exp"] = exp_pool.tile([128, 1536], bf16, tag="exp",
                                                 name=f"exp_{u}")
                s = state["units"]
                tp = (96, 0) if hq == 3 else None
                nc.tensor.matmul(
                    state["psum"][:, s * 512:(s + 1) * 512],
                    kb[Q][32 * hq:32 * (hq + 1), jc * 128:(jc + 1) * 128],
                    qb[Q][32 * hq:32 * (hq + 1), ih * 512:(ih + 1) * 512],
                    start=True,
                    stop=True,
                    tile_position=tp,
                )
                exp_slices[(g, pq, jc, which)] = (state["exp"], s)
                state["units"] += 1
                if state["units"] == 3:
                    flush_ring()
            if last:
                flush_ring()

        # ---- AV + normalize ----
        def av_pair(g, pq):
            ih, Q = g // 2, g % 2
            hA, hB = 4 * Q + 2 * pq, 4 * Q + 2 * pq + 1  # global heads
            ps = acc_psum.tile([128, 512], f32, tag="acc", name=f"av_{g}_{pq}")
            for jc in range(8):
                eA, sA = exp_slices[(g, pq, jc, 0)]
                eB, sB = exp_slices[(g, pq, jc, 1)]
                st, sp = (jc == 0), (jc == 7)
                nc.tensor.matmul(
                    ps[0:64, :], vt[jc][:, 64 * hA:64 * hA + 64],
                    eA[:, sA * 512:(sA + 1) * 512], start=st, stop=sp)
                nc.tensor.matmul(
                    ps[64:128, :], vt[jc][:, 64 * hB:64 * hB + 64],
                    eB[:, sB * 512:(sB + 1) * 512], start=st, stop=sp,
                    tile_position=(0, 64))
            # rows 0:32 mainA, 32:64 denA, 64:96 mainB, 96:128 denB
            rc = rc_pool.tile([128, 512], f32, tag="rc")
            nc.vector.reciprocal_approx_fast(out=rc[:, :], in_=ps[:, :])
            dst = out_allT[Q]
            r0 = 64 * pq
            nc.vector.tensor_mul(
                out=dst[r0:r0 + 32, ih * 512:(ih + 1) * 512],
                in0=ps[0:32, :], in1=rc[32:64, :])
            nc.vector.tensor_mul(
                out=dst[r0 + 32:r0 + 64, ih * 512:(ih + 1) * 512],
                in0=ps[64:96, :], in1=rc[96:128, :])

        def emit_proj(nh, oc, dma_engine, bias_on_act=False):
            ps = acc_psum.tile([128, 512], f32, tag="acc",
                               name=f"proj_{nh}_{oc}")
            for t in range(2):
                nc.tensor.matmul(
                    ps,
                    wot[:, 256 * t + 128 * oc:256 * t + 128 * oc + 128],
                    out_allT[t][:, nh * 512:(nh + 1) * 512],
                    start=(t == 0),
                    stop=(t == 1),
                )
            ys = evac.tile([128, 512], bf16, tag="y")
            if bias_on_act:
                # tail only: ACT is idle once the last exp has issued
                nc.scalar.add(out=ys, in_=ps, add=bias2[:, oc:oc + 1])
            else:
                nc.vector.tensor_scalar_add(out=ys, in0=ps,
                                            scalar1=bias2[:, oc:oc + 1])
            dma_engine.dma_start(
                out=y_ap[oc * 128:(oc + 1) * 128, nh * 512:(nh + 1) * 512],
                in_=ys,
            )

        # ---- emission schedule (block-contiguous, half-group granularity) ----
        emit_qkv_chunk("k", 0, 0)
        emit_qkv_chunk("q", 0, 0)
        emit_qkv_chunk("k", 0, 1)
        emit_qkv_chunk("q", 0, 1)
        emit_sim_units(0, 0, 16)
        emit_qkv_chunk("k", 1, 0)
        emit_qkv_chunk("k", 1, 1)
        emit_qkv_chunk("q", 1, 0)
        emit_qkv_chunk("q", 1, 1)
        emit_sim_units(0, 16, 32)
        for jc in range(8):
            emit_vt(jc)
        emit_sim_units(1, 0, 32)
        av_pair(0, 0)
        av_pair(0, 1)
        emit_sim_units(2, 0, 32)
        av_pair(1, 0)
        av_pair(1, 1)
        # proj of token half 0: out_allT[*][:, 0:512] complete (groups 0+1)
        emit_proj(0, 0, nc.sync)
        emit_proj(0, 1, nc.sync)
        emit_sim_units(3, 0, 16)       # pairB units first (unit_pq flip)
        av_pair(2, 0)
        emit_sim_units(3, 16, 24)
        av_pair(2, 1)
        emit_sim_units(3, 24, 32, last=True)
        av_pair(3, 1)                  # units were w 0..15: exp'd early
        av_pair(3, 0)                  # tail: waits the final rings
        emit_proj(1, 0, nc.scalar, bias_on_act=True)
        emit_proj(1, 1, nc.sync)

        if dbg is not None:
            for nm, tile_ in (("qb0", qb[0]), ("kb0", kb[0]), ("vt0", vt[0]),
                              ("oa0", out_allT[0]), ("oa1", out_allT[1])):
                if nm in dbg:
                    nc.sync.dma_start(out=dbg[nm], in_=tile_)
            if "exp0" in dbg:
                et, s = exp_slices[(0, 0)]
                nc.sync.dma_start(out=dbg["exp0"], in_=et[:, s * 512:(s + 1) * 512])


def _prep_weights(w_qkv, w_out, b_out):
    """Host-side weight preparation (numpy)."""
    wq = w_qkv.astype(np.float32).copy()
    wq[0:DIM_HEAD] *= SCALE                      # fold softmax scale into w_q
    wqkvT = np.ascontiguousarray(wq.T)                        # [256, 768]
    # per-half column order [k | q | v] so the k/q chunks DMA first
    wqkvT = np.concatenate(
        [wqkvT[:, 256:512], wqkvT[:, 0:256], wqkvT[:, 512:768]], axis=1)
    wqkvT = np.concatenate([wqkvT[0:128], wqkvT[128:256]], axis=1)  # [128,1536]
    wqkvT = np.ascontiguousarray(wqkvT).astype(_BF16)
    woutT = np.ascontiguousarray(w_out.astype(np.float32).T)  # [256, 256]
    woutT = np.ascontiguousarray(
        np.concatenate([woutT[0:128], woutT[128:256]], axis=1)).astype(_BF16)
    bout = np.ascontiguousarray(
        b_out.astype(np.float32).reshape(2, 128).T)           # [128, 2]
    return wqkvT, woutT, bout


def _build_program():
    global _PROGRAM
    if _PROGRAM is not None:
        return _PROGRAM
    import concourse.tile as tile
    from concourse import bacc, mybir

    nc = bacc.Bacc("TRN2", target_bir_lowering=False, debug=False,
                   num_devices=NCORES)
    x_ap = nc.dram_tensor("x", [128, 2048], mybir.dt.bfloat16,
                          kind="ExternalInput").ap()
    wqkvT_ap = nc.dram_tensor("wqkvT", [128, 1536], mybir.dt.bfloat16,
                              kind="ExternalInput").ap()
    woutT_ap = nc.dram_tensor("woutT", [128, 512], mybir.dt.bfloat16,
                              kind="ExternalInput").ap()
    bout_ap = nc.dram_tensor("bout", [128, 2], mybir.dt.float32,
                             kind="ExternalInput").ap()
    y_ap = nc.dram_tensor("y", [DIM, N], mybir.dt.bfloat16,
                          kind="ExternalOutput").ap()
    with tile.TileContext(nc) as tc:
        build_kernel_body(tc, y_ap, x_ap, wqkvT_ap, woutT_ap, bout_ap)
    nc.compile()
    _PROGRAM = nc
    return nc


def kernel(x, w_qkv, w_out, b_out, trace=False):
    """Full-input entry point: shard over batch, run on 8 cores, gather."""
    from concourse import bass_utils

    nc = _build_program()
    wqkvT, woutT, bout = _prep_weights(w_qkv, w_out, b_out)
    in_maps = []
    for b in range(B):
        xb = np.asarray(x[b], dtype=np.float32).reshape(DIM, N)
        xb = np.ascontiguousarray(
            np.concatenate([xb[0:128], xb[128:256]], axis=1)).astype(_BF16)
        in_maps.append({
            "x": xb,
            "wqkvT": wqkvT,
            "woutT": woutT,
            "bout": bout,
        })
    res = bass_utils.run_bass_kernel_spmd(
        nc, in_maps, core_ids=list(range(NCORES)), trace=trace)
    y = np.stack([
        res.results[b]["y"].astype(np.float32).reshape(DIM, H, W)
        for b in range(B)
    ])
    kernel.last_results = res
    return y
